# revision 8
# baseline (speedup 1.0000x reference)
"""Multi-head attention forward on 8 Trainium2 NeuronCores.

Strategy: pure data-parallel over batch (B=8 -> 1 batch element per core,
no collectives). Per core, one fused kernel computes
    y = softmax((x Wq + bq)(x Wk + bk)^T / sqrt(hd)) (x Wv + bv) @ Wp + bp
for x [1024, 768], H=12 heads of 64 dims.

v2 pipeline (ScalarE exp stream is the heartbeat at ~1.1us per
[128,1024] tile, 96 tiles total ~107us; everything else hides under it):
  - x^T via PE transposes; x f32 DMA on the sync queue, f32->bf16 cast on
    ScalarE (idle during startup), PSUM drain on DVE.
  - W_qkv DMA'd in column slices (gpsimd cast-DMA f32->bf16), ordered
    pair-0 Q/K first, then V, then the remaining pairs, then W_proj --
    pair-0 QKV matmuls start ~11us in instead of waiting for the full W.
  - Per stage g: scores for pair g emitted kb0/kb1 first (keeps ScalarE
    fed across stage boundaries), then per-kb: interleaved fillers
    (prev-pair AV chains + AV epilogues/norm + V-gen + next-pair QKV +
    proj partials) paced between score matmuls.
  - Scores row-tiled 2x on the PE (K=64 pairs at base partitions 0/64).
  - AV per head: lhsT = V_ext [128, 65] (ones column -> softmax sums Z in
    row 64), accumulated over kb in PSUM.
  - Normalization: 1/Z on DVE (reciprocal_approx_fast) straight from the
    AV PSUM row 64, broadcast to 64 rows with K=1 matmuls from partition
    64, one DVE multiply per (pair, q-half). No ScalarE, no DRAM staging.
  - proj: pairs 0-4 accumulate into PSUM during stage 5's exp window,
    drained bf16 to SBUF; pair 5 + tail adds finish after the last exp.
Compute dtype bf16 (fp32 PSUM accumulation).
"""

import sys

for _p in ("/opt/trn_rl_repo", "/root/.axon_site/_ro/trn_rl_repo"):
    if _p not in sys.path:
        sys.path.append(_p)

import numpy as np

import concourse.bacc as bacc
import concourse.mybir as mybir
import concourse.tile as tile
from concourse.bass_utils import run_bass_kernel_spmd
from concourse.masks import make_identity

N_CORES = 8
P = 128
S = 1024
D = 768
H = 12
HD = 64
ND = D // P            # 6 d_model chunks
NS = S // P            # 8 seq tiles
NM = (2 * D) // P      # 12 M-tiles over Q,K douts
SCALE = 1.0 / (HD ** 0.5)
BF = mybir.dt.bfloat16
F32 = mybir.dt.float32
AF = mybir.ActivationFunctionType
ALU = mybir.AluOpType

_cached = None


def _patch_act_tables():
    """Force every Exp activation onto one table set so the table is
    loaded once."""
    import concourse.bacc as _bacc
    if getattr(_bacc, "_act_tables_patched", False):
        return
    orig = _bacc.get_activation_tables

    def patched(arch):
        tables = dict(orig(arch))
        for name, fns in tables.items():
            if name != "natural_log_exp_and_others":
                tables[name] = fns - {AF.Exp, AF.Ln}
        return tables

    _bacc.get_activation_tables = patched
    _bacc._act_tables_patched = True


def _build():
    _patch_act_tables()
    nc = bacc.Bacc("TRN2", target_bir_lowering=False, debug=False,
                   enable_asserts=True, num_devices=N_CORES)

    x_ext = nc.dram_tensor("x", [S, D], F32, kind="ExternalInput").ap()
    wq_ext = nc.dram_tensor("W_qkv", [D, 3 * D], F32, kind="ExternalInput").ap()
    bq_ext = nc.dram_tensor("b_qkv", [1, 3 * D], F32, kind="ExternalInput").ap()
    wp_ext = nc.dram_tensor("W_proj", [D, D], F32, kind="ExternalInput").ap()
    bp_ext = nc.dram_tensor("b_proj", [1, D], F32, kind="ExternalInput").ap()
    out_ext = nc.dram_tensor("out", [S, D], F32, kind="ExternalOutput").ap()

    with tile.TileContext(nc) as tc:
        _body(nc, tc, x_ext, wq_ext, bq_ext, wp_ext, bp_ext, out_ext)

    nc.compile()
    return nc


def _body(nc, tc, x_ext, wq_ext, bq_ext, wp_ext, bp_ext, out_ext):
    from contextlib import ExitStack
    from concourse.tile import add_dep_helper
    import itertools

    with ExitStack() as ctx:
        persist = ctx.enter_context(tc.tile_pool(name="persist", bufs=1))
        yout = ctx.enter_context(tc.tile_pool(name="yout", bufs=2))
        ps_mm = ctx.enter_context(tc.tile_pool(name="ps_mm", bufs=2, space="PSUM"))

        ident = persist.tile([P, P], BF)
        make_identity(nc, ident)
        xT = persist.tile([P, ND, S], BF)

        # ---- constants / biases ----
        bqkT = persist.tile([P, NM], F32)   # col m = b_qkv[m*128:(m+1)*128]
        for m in range(NM):
            nc.sync.dma_start(bqkT[:, m:m + 1], bq_ext[0:1, m * P:(m + 1) * P])
        bv_bf = persist.tile([1, D], BF)
        nc.gpsimd.dma_start(bv_bf, bq_ext[0:1, 2 * D:3 * D])
        bp_bf = persist.tile([1, D], BF)
        nc.gpsimd.dma_start(bp_bf, bp_ext[0:1, :])
        ones1 = persist.tile([1, P], BF)
        nc.vector.memset(ones1, 1.0)
        ones64 = persist.tile([65, HD], BF)
        nc.vector.memset(ones64[64:65, :], 1.0)
        vext = persist.tile([P, NS, H * 65], BF)
        for sb in range(NS):
            vd = vext[:, sb, :].rearrange("p (h c) -> p h c", c=65)
            nc.vector.memset(vd[:, :, 64:65], 1.0)

        # ---- weights: gpsimd cast-DMA f32->bf16, column-sliced, ordered
        # so pair-0 Q/K land first, then V, then the rest ----
        w_bf = persist.tile([P, ND, 3 * D], BF)
        wp_bf = persist.tile([P, ND, D], BF)     # row chunk g = head pair g

        def w_cols(c0, cn):
            src = wq_ext[0:D, c0:c0 + cn].rearrange("(kc p) c -> p kc c", p=P)
            return nc.gpsimd.dma_start(w_bf[:, :, c0:c0 + cn], src)

        w_chain = w_cols(0 * P, P)                       # Q pair 0
        for c0, cn in [(D + 0 * P, P),                   # K pair 0
                       (2 * D, 384), (2 * D + 384, 384), # V (both halves)
                       (1 * P, P), (D + 1 * P, P),       # pair 1
                       (2 * P, P), (D + 2 * P, P),
                       (3 * P, P), (D + 3 * P, P),
                       (4 * P, P), (D + 4 * P, P),
                       (5 * P, P), (D + 5 * P, P)]:
            nxt = w_cols(c0, cn)
            add_dep_helper(nxt.ins, w_chain.ins, reason="w slice order")
            w_chain = nxt
        for g in range(ND):
            wp_dma = nc.gpsimd.dma_start(wp_bf[:, g, :],
                                         wp_ext[g * P:(g + 1) * P, :])
            add_dep_helper(wp_dma.ins, w_chain.ins,
                           reason="proj weights after qkv weights")

        # ---- x: f32 DMA (sync queue), cast on ScalarE, PE transpose,
        # DVE drain ----
        with tc.tile_pool(name="xin", bufs=3) as xin, \
             tc.tile_pool(name="ps_tr", bufs=6, space="PSUM") as ps_tr:
            for sb in range(NS):
                x_f = xin.tile([P, D], F32, tag="x_f")
                nc.sync.dma_start(x_f[:, 0:D // 2],
                                  x_ext[sb * P:(sb + 1) * P, 0:D // 2])
                nc.sync.dma_start(x_f[:, D // 2:D],
                                  x_ext[sb * P:(sb + 1) * P, D // 2:D])
                x_b = xin.tile([P, D], BF, tag="x_b")
                nc.scalar.activation(x_b[:, 0:D // 2], x_f[:, 0:D // 2],
                                     AF.Copy)
                nc.scalar.activation(x_b[:, D // 2:D], x_f[:, D // 2:D],
                                     AF.Copy)
                for kc in range(ND):
                    pt = ps_tr.tile([P, P], BF, tag="ps_tr")
                    nc.tensor.transpose(pt, x_b[:, kc * P:(kc + 1) * P], ident)
                    nc.vector.tensor_copy(xT[:, kc, sb * P:(sb + 1) * P], pt)

        expp = ctx.enter_context(tc.tile_pool(name="expp", bufs=28))
        sums_p = ctx.enter_context(tc.tile_pool(name="sums", bufs=1))
        ps_sc = ctx.enter_context(tc.tile_pool(name="ps_sc", bufs=2, space="PSUM"))
        ps_av = ctx.enter_context(tc.tile_pool(name="ps_av", bufs=2, space="PSUM"))

        qkT = persist.tile([P, NM, S], BF)
        aoT = persist.tile([P, ND, S], BF)   # paired attn out^T: pair g rows
        y01 = persist.tile([P, NS, D], BF)   # proj partial (pairs 0-4 + bias)

        # ---------------- generators (emission-paced fillers) ----------
        def gen_qkT(g):
            """Q^T/K^T tiles for pair g, one yield per PE instruction."""
            for m, nh in ((g, 0), (ND + g, 0), (g, 1), (ND + g, 1)):
                ps = ps_mm.tile([P, 512], F32, tag="ps_mm",
                                name=f"qk{m}_{nh}")
                for kc in range(ND):
                    nc.tensor.matmul(ps,
                                     w_bf[:, kc, m * P:(m + 1) * P],
                                     xT[:, kc, nh * 512:(nh + 1) * 512],
                                     start=(kc == 0), stop=(kc == ND - 1))
                    yield
                nc.vector.tensor_scalar(
                    out=qkT[:, m, nh * 512:(nh + 1) * 512], in0=ps,
                    scalar1=bqkT[:, m:m + 1], scalar2=None, op0=ALU.add)
                yield

        def gen_v(sb0, sb1):
            """V rows for seq blocks [sb0, sb1): s-major with the softmax
            ones column per head."""
            for sb in range(sb0, sb1):
                for c0, cn in ((0, 512), (512, 256)):
                    ps = ps_mm.tile([P, 512], F32, tag="ps_mm",
                                    name=f"v{sb}_{c0}")
                    for kc in range(ND):
                        nc.tensor.matmul(ps[:, :cn],
                                         xT[:, kc, sb * P:(sb + 1) * P],
                                         w_bf[:, kc, 2 * D + c0:2 * D + c0 + cn],
                                         start=(kc == 0), stop=False)
                        yield
                    nc.tensor.matmul(ps[:, :cn], ones1, bv_bf[:, c0:c0 + cn],
                                     start=False, stop=True)
                    yield
                    h0 = c0 // HD
                    nh_h = cn // HD
                    vsrc = ps[:, :cn].rearrange("p (h c) -> p h c", c=HD)
                    vdst = vext[:, sb, :].rearrange("p (h c) -> p h c", c=65)
                    nc.vector.tensor_copy(vdst[:, h0:h0 + nh_h, 0:HD], vsrc)
                    yield

        def av_chain(g, half, qh):
            """Full AV accumulation for (pair g, head half, q half) plus
            its epilogue: 1/Z on DVE from PSUM row 64, attn-out rows to
            aoT. Yields per PE instruction; returns rzb via a dict."""
            h = 2 * g + half
            qs = slice(qh * 512, (qh + 1) * 512)
            po = ps_av.tile([65, 512], F32, tag="ps_av", name=f"po{h}_{qh}")
            exps = stage_exps[g]
            for kb in range(NS):
                nc.tensor.matmul(po,
                                 vext[:, kb, h * 65:(h + 1) * 65],
                                 exps[half][kb][:, qs],
                                 start=(kb == 0), stop=(kb == NS - 1))
                yield
            rows = slice(half * HD, (half + 1) * HD)
            zs = sums_p.tile([65, 512], F32, tag="zs", bufs=2,
                             name=f"zs{h}_{qh}")
            nc.vector.tensor_copy(zs[64:65, :], po[64:65, :])
            rz = sums_p.tile([65, 512], F32, tag="rz", bufs=2,
                             name=f"rz{h}_{qh}")
            nc.vector.reciprocal(rz[64:65, :], zs[64:65, :])
            rzb = sums_p.tile([65, 512], BF, tag="rzb", bufs=4,
                              name=f"rzb{h}_{qh}")
            nc.vector.tensor_copy(rzb[64:65, :], rz[64:65, :])
            nc.vector.tensor_copy(aoT[rows, g, qs], po[0:64, :])
            rzbs[(g, half, qh)] = rzb
            yield

        def norm_qh(g, qh):
            """Broadcast 1/Z to 64 rows per head (K=1 matmuls from
            partition 64) and scale aoT for (pair g, q half)."""
            qs = slice(qh * 512, (qh + 1) * 512)
            pb = ps_mm.tile([P, 512], F32, tag="ps_mm", name=f"pb{g}_{qh}")
            for half in range(2):
                rows = slice(half * HD, (half + 1) * HD)
                rzb = rzbs[(g, half, qh)]
                nc.tensor.matmul(pb[rows, :], ones64[64:65, :],
                                 rzb[64:65, :], start=True, stop=True)
                yield
            nc.vector.tensor_mul(aoT[:, g, qs], aoT[:, g, qs], pb)
            yield

        def av_and_norm(g, halves_qh):
            """Chain AV accumulation + normalization for the given
            (half, qh) combos of pair g."""
            done_qh = set()
            for half, qh in halves_qh:
                for _ in av_chain(g, half, qh):
                    yield
                if (g, 0, qh) in rzbs and (g, 1, qh) in rzbs \
                        and qh not in done_qh:
                    done_qh.add(qh)
                    for _ in norm_qh(g, qh):
                        yield

        def gen_proj_partial(sb_list):
            """proj partial sums over pairs 0-4 plus bias, drained bf16 to
            y01. Runs in stage 5's exp window (ps_mm pool is free of QKV
            work by then)."""
            for sb in sb_list:
                pj0 = ps_mm.tile([P, 512], F32, tag="ps_mm", name=f"pj0_{sb}")
                pj1 = ps_mm.tile([P, 512], F32, tag="ps_mm", name=f"pj1_{sb}")
                for g in range(ND - 1):
                    nc.tensor.matmul(pj0,
                                     aoT[:, g, sb * P:(sb + 1) * P],
                                     wp_bf[:, g, 0:512],
                                     start=(g == 0), stop=False)
                    yield
                    nc.tensor.matmul(pj1[:, 0:256],
                                     aoT[:, g, sb * P:(sb + 1) * P],
                                     wp_bf[:, g, 512:768],
                                     start=(g == 0), stop=False)
                    yield
                nc.tensor.matmul(pj0, ones1, bp_bf[:, 0:512],
                                 start=False, stop=True)
                nc.tensor.matmul(pj1[:, 0:256], ones1, bp_bf[:, 512:768],
                                 start=False, stop=True)
                yield
                nc.vector.tensor_copy(y01[:, sb, 0:512], pj0)
                nc.vector.tensor_copy(y01[:, sb, 512:768], pj1[:, 0:256])
                yield

        # ---------------- stages ----------------
        stage_exps = {}
        rzbs = {}

        def stage(g, fillers, n_fill=7, n_front=2):
            """Scores+exp for pair g; the first n_front kbs are emitted
            score-first so ScalarE never starves at stage boundaries;
            fillers are paced between the remaining kbs."""
            e0 = []
            e1 = []
            stage_exps[g] = (e0, e1)
            for kb in range(NS):
                if kb >= n_front:
                    for _ in range(n_fill):
                        if next(fillers, None) is None:
                            break
                ps0 = ps_sc.tile([P, S], F32, tag="ps_sc", name=f"sc0_{g}_{kb}")
                ps1 = ps_sc.tile([P, S], F32, tag="ps_sc", name=f"sc1_{g}_{kb}")
                for qh in range(2):
                    qs = slice(qh * 512, (qh + 1) * 512)
                    nc.tensor.matmul(ps0[:, qs],
                                     qkT[0:HD, ND + g, kb * P:(kb + 1) * P],
                                     qkT[0:HD, g, qs], start=True, stop=True)
                    nc.tensor.matmul(ps1[:, qs],
                                     qkT[HD:P, ND + g, kb * P:(kb + 1) * P],
                                     qkT[HD:P, g, qs], start=True, stop=True)
                t0 = expp.tile([P, S], BF, tag="expT", name=f"e0_{g}_{kb}")
                t1 = expp.tile([P, S], BF, tag="expT", name=f"e1_{g}_{kb}")
                nc.scalar.activation(t0, ps0, AF.Exp, scale=SCALE)
                nc.scalar.activation(t1, ps1, AF.Exp, scale=SCALE)
                e0.append(t0)
                e1.append(t1)
            return fillers

        ALL_COMBOS = ((0, 0), (0, 1), (1, 0), (1, 1))

        # emit pair-0 Q/K immediately (gated only on x + first W slices)
        for _ in gen_qkT(0):
            pass

        # stage 0: scores(0); fillers: V sb0-4 + qkT(1)
        f = stage(0, itertools.chain(gen_v(0, 5), gen_qkT(1)), n_fill=19)
        for _ in f:
            pass
        # stage 1: scores(1); fillers: V sb5-7 + qkT(2) + AV(0)
        # (V before the norm pb tiles: both use the 2-buf ps_mm ring, and
        # AV depends on V, so V's tiles must be allocated first)
        f = stage(1, itertools.chain(gen_v(5, NS), gen_qkT(2),
                                     av_and_norm(0, ALL_COMBOS)), n_fill=21)
        for _ in f:
            pass
        # stages 2-4: scores(g); fillers: qkT(g+1) + AV(g-1)
        for g in range(2, ND - 1):
            f = stage(g, itertools.chain(gen_qkT(g + 1),
                                         av_and_norm(g - 1, ALL_COMBOS)),
                      n_fill=13)
            for _ in f:
                pass
        # stage 5: scores(5); fillers: AV(4) + proj partials (pairs 0-4);
        # AV(5) runs in the post-loop drain (its exps only exist then)
        f = stage(ND - 1,
                  itertools.chain(av_and_norm(ND - 2, ALL_COMBOS),
                                  gen_proj_partial(range(NS)),
                                  av_and_norm(ND - 1, ALL_COMBOS)),
                  n_fill=20)
        for _ in f:
            pass

        # ---- tail: pair-5 proj contribution + final add + store ----
        for sb in range(NS):
            g = ND - 1
            pt0 = ps_mm.tile([P, 512], F32, tag="ps_mm", name=f"pt0_{sb}")
            pt1 = ps_mm.tile([P, 512], F32, tag="ps_mm", name=f"pt1_{sb}")
            nc.tensor.matmul(pt0, aoT[:, g, sb * P:(sb + 1) * P],
                             wp_bf[:, g, 0:512], start=True, stop=True)
            nc.tensor.matmul(pt1[:, 0:256], aoT[:, g, sb * P:(sb + 1) * P],
                             wp_bf[:, g, 512:768], start=True, stop=True)
            y_sb = yout.tile([P, D], F32, tag="y")
            nc.vector.tensor_tensor(y_sb[:, 0:512], pt0, y01[:, sb, 0:512],
                                    op=ALU.add)
            nc.vector.tensor_tensor(y_sb[:, 512:768], pt1[:, 0:256],
                                    y01[:, sb, 512:768], op=ALU.add)
            eng = nc.sync if sb % 2 == 0 else nc.scalar
            eng.dma_start(out_ext[sb * P:(sb + 1) * P, :], y_sb)


def kernel(**inputs):
    global _cached
    x = np.ascontiguousarray(np.asarray(inputs["x"], dtype=np.float32))
    w_qkv = np.ascontiguousarray(np.asarray(inputs["W_qkv"], dtype=np.float32))
    b_qkv = np.ascontiguousarray(np.asarray(inputs["b_qkv"], dtype=np.float32)).reshape(1, -1)
    w_proj = np.ascontiguousarray(np.asarray(inputs["W_proj"], dtype=np.float32))
    b_proj = np.ascontiguousarray(np.asarray(inputs["b_proj"], dtype=np.float32)).reshape(1, -1)

    if _cached is None:
        _cached = _build()
    nc = _cached

    in_maps = [{"x": x[b], "W_qkv": w_qkv, "b_qkv": b_qkv,
                "W_proj": w_proj, "b_proj": b_proj} for b in range(N_CORES)]
    last_err = None
    for _attempt in range(3):
        try:
            res = run_bass_kernel_spmd(nc, in_maps,
                                       core_ids=list(range(N_CORES)))
            return np.stack([res.results[i]["out"] for i in range(N_CORES)],
                            axis=0)
        except Exception as e:  # transient NRT device errors happen rarely
            last_err = e
            import time
            time.sleep(2.0)
    raise last_err


# revision 11
# speedup vs baseline: 1.1897x; 1.1897x over previous
"""Multi-head attention forward on 8 Trainium2 NeuronCores.

Strategy: pure data-parallel over batch (B=8 -> 1 batch element per core,
no collectives). Per core, one fused kernel computes
    y = softmax((x Wq + bq)(x Wk + bk)^T / sqrt(hd)) (x Wv + bv) @ Wp + bp
for x [1024, 768], H=12 heads of 64 dims.

v2 pipeline (ScalarE exp stream is the heartbeat at ~1.1us per
[128,1024] tile, 96 tiles total ~107us; everything else hides under it):
  - x^T via PE transposes; x f32 DMA on the sync queue, f32->bf16 cast on
    ScalarE (idle during startup), PSUM drain on DVE.
  - W_qkv DMA'd in column slices (gpsimd cast-DMA f32->bf16), ordered
    pair-0 Q/K first, then V, then the remaining pairs, then W_proj --
    pair-0 QKV matmuls start ~11us in instead of waiting for the full W.
  - Per stage g: scores for pair g emitted kb0/kb1 first (keeps ScalarE
    fed across stage boundaries), then per-kb: interleaved fillers
    (prev-pair AV chains + AV epilogues/norm + V-gen + next-pair QKV +
    proj partials) paced between score matmuls.
  - Scores row-tiled 2x on the PE (K=64 pairs at base partitions 0/64).
  - AV per head: lhsT = V_ext [128, 65] (ones column -> softmax sums Z in
    row 64), accumulated over kb in PSUM.
  - Normalization: 1/Z on DVE (reciprocal_approx_fast) straight from the
    AV PSUM row 64, broadcast to 64 rows with K=1 matmuls from partition
    64, one DVE multiply per (pair, q-half). No ScalarE, no DRAM staging.
  - proj: pairs 0-4 accumulate into PSUM during stage 5's exp window,
    drained bf16 to SBUF; pair 5 + tail adds finish after the last exp.
Compute dtype bf16 (fp32 PSUM accumulation).
"""

import sys

for _p in ("/opt/trn_rl_repo", "/root/.axon_site/_ro/trn_rl_repo"):
    if _p not in sys.path:
        sys.path.append(_p)

import numpy as np

import concourse.bacc as bacc
import concourse.mybir as mybir
import concourse.tile as tile
from concourse.bass_utils import run_bass_kernel_spmd
from concourse.masks import make_identity

N_CORES = 8
P = 128
S = 1024
D = 768
H = 12
HD = 64
ND = D // P            # 6 d_model chunks
NS = S // P            # 8 seq tiles
NM = (2 * D) // P      # 12 M-tiles over Q,K douts
SCALE = 1.0 / (HD ** 0.5)
BF = mybir.dt.bfloat16
F32 = mybir.dt.float32
AF = mybir.ActivationFunctionType
ALU = mybir.AluOpType

_cached = None


def _patch_act_tables():
    """Force every Exp activation onto one table set so the table is
    loaded once."""
    import concourse.bacc as _bacc
    if getattr(_bacc, "_act_tables_patched", False):
        return
    orig = _bacc.get_activation_tables

    def patched(arch):
        tables = dict(orig(arch))
        for name, fns in tables.items():
            if name != "natural_log_exp_and_others":
                tables[name] = fns - {AF.Exp, AF.Ln}
        return tables

    _bacc.get_activation_tables = patched
    _bacc._act_tables_patched = True


def _build():
    _patch_act_tables()
    nc = bacc.Bacc("TRN2", target_bir_lowering=False, debug=False,
                   enable_asserts=True, num_devices=N_CORES)

    x_ext = nc.dram_tensor("x", [S, D], F32, kind="ExternalInput").ap()
    wq_ext = nc.dram_tensor("W_qkv", [D, 3 * D], F32, kind="ExternalInput").ap()
    bq_ext = nc.dram_tensor("b_qkv", [1, 3 * D], F32, kind="ExternalInput").ap()
    wp_ext = nc.dram_tensor("W_proj", [D, D], F32, kind="ExternalInput").ap()
    bp_ext = nc.dram_tensor("b_proj", [1, D], F32, kind="ExternalInput").ap()
    out_ext = nc.dram_tensor("out", [S, D], F32, kind="ExternalOutput").ap()

    with tile.TileContext(nc) as tc:
        _body(nc, tc, x_ext, wq_ext, bq_ext, wp_ext, bp_ext, out_ext)

    nc.compile()
    return nc


def _body(nc, tc, x_ext, wq_ext, bq_ext, wp_ext, bp_ext, out_ext):
    from contextlib import ExitStack
    from concourse.tile import add_dep_helper
    import itertools

    with ExitStack() as ctx:
        persist = ctx.enter_context(tc.tile_pool(name="persist", bufs=1))
        yout = ctx.enter_context(tc.tile_pool(name="yout", bufs=2))
        ps_mm = ctx.enter_context(tc.tile_pool(name="ps_mm", bufs=2, space="PSUM"))

        ident = persist.tile([P, P], BF)
        make_identity(nc, ident)
        xT = persist.tile([P, ND, S], BF)

        # ---- constants / biases ----
        bqkT = persist.tile([P, NM], F32)   # col m = b_qkv[m*128:(m+1)*128]
        for m in range(NM):
            nc.sync.dma_start(bqkT[:, m:m + 1], bq_ext[0:1, m * P:(m + 1) * P])
        bv_bf = persist.tile([1, D], BF)
        nc.gpsimd.dma_start(bv_bf, bq_ext[0:1, 2 * D:3 * D])
        bp_bf = persist.tile([1, D], BF)
        nc.gpsimd.dma_start(bp_bf, bp_ext[0:1, :])
        ones1 = persist.tile([1, P], BF)
        nc.vector.memset(ones1, 1.0)
        ones4 = persist.tile([97, HD], BF)
        for r in (0, 32, 64, 96):
            nc.vector.memset(ones4[r:r + 1, :], 1.0)
        vext = persist.tile([P, NS, H * 65], BF)
        for sb in range(NS):
            vd = vext[:, sb, :].rearrange("p (h c) -> p h c", c=65)
            nc.vector.memset(vd[:, :, 64:65], 1.0)

        # ---- weights: gpsimd cast-DMA f32->bf16, column-sliced, ordered
        # so pair-0 Q/K land first, then V, then the rest ----
        w_bf = persist.tile([P, ND, 3 * D], BF)
        wp_bf = persist.tile([P, ND, D], BF)     # row chunk g = head pair g

        def w_cols(c0, cn):
            src = wq_ext[0:D, c0:c0 + cn].rearrange("(kc p) c -> p kc c", p=P)
            return nc.gpsimd.dma_start(w_bf[:, :, c0:c0 + cn], src)

        w_chain = w_cols(0 * P, P)                       # Q pair 0
        for c0, cn in [(D + 0 * P, P),                   # K pair 0
                       (2 * D, 384), (2 * D + 384, 384), # V (both halves)
                       (1 * P, P), (D + 1 * P, P),       # pair 1
                       (2 * P, P), (D + 2 * P, P),
                       (3 * P, P), (D + 3 * P, P),
                       (4 * P, P), (D + 4 * P, P),
                       (5 * P, P), (D + 5 * P, P)]:
            nxt = w_cols(c0, cn)
            add_dep_helper(nxt.ins, w_chain.ins, reason="w slice order")
            w_chain = nxt
        for g in range(ND):
            wp_dma = nc.gpsimd.dma_start(wp_bf[:, g, :],
                                         wp_ext[g * P:(g + 1) * P, :])
            add_dep_helper(wp_dma.ins, w_chain.ins,
                           reason="proj weights after qkv weights")

        # ---- x: f32 DMA (sync queue), cast on ScalarE, PE transpose,
        # DVE drain ----
        with tc.tile_pool(name="xin", bufs=3) as xin, \
             tc.tile_pool(name="ps_tr", bufs=6, space="PSUM") as ps_tr:
            for sb in range(NS):
                x_f = xin.tile([P, D], F32, tag="x_f")
                nc.sync.dma_start(x_f[:, 0:D // 2],
                                  x_ext[sb * P:(sb + 1) * P, 0:D // 2])
                nc.sync.dma_start(x_f[:, D // 2:D],
                                  x_ext[sb * P:(sb + 1) * P, D // 2:D])
                x_b = xin.tile([P, D], BF, tag="x_b")
                nc.scalar.activation(x_b[:, 0:D // 2], x_f[:, 0:D // 2],
                                     AF.Copy)
                nc.scalar.activation(x_b[:, D // 2:D], x_f[:, D // 2:D],
                                     AF.Copy)
                for kc in range(ND):
                    pt = ps_tr.tile([P, P], BF, tag="ps_tr")
                    nc.tensor.transpose(pt, x_b[:, kc * P:(kc + 1) * P], ident)
                    nc.vector.tensor_copy(xT[:, kc, sb * P:(sb + 1) * P], pt)

        expp = ctx.enter_context(tc.tile_pool(name="expp", bufs=28))
        sums_p = ctx.enter_context(tc.tile_pool(name="sums", bufs=1))
        ps_sc = ctx.enter_context(tc.tile_pool(name="ps_sc", bufs=2, space="PSUM"))
        ps_av = ctx.enter_context(tc.tile_pool(name="ps_av", bufs=2, space="PSUM"))

        qkT = persist.tile([P, NM, S], BF)
        aoT = persist.tile([P, ND, S], BF)   # paired attn out^T: pair g rows
        y01 = persist.tile([P, NS, D], BF)   # proj partial (pairs 0-4 + bias)

        # ---------------- generators (emission-paced fillers) ----------
        def gen_qkT(g):
            """Q^T/K^T tiles for pair g, one yield per PE instruction."""
            for m, nh in ((g, 0), (ND + g, 0), (g, 1), (ND + g, 1)):
                ps = ps_mm.tile([P, 512], F32, tag="ps_mm",
                                name=f"qk{m}_{nh}")
                for kc in range(ND):
                    nc.tensor.matmul(ps,
                                     w_bf[:, kc, m * P:(m + 1) * P],
                                     xT[:, kc, nh * 512:(nh + 1) * 512],
                                     start=(kc == 0), stop=(kc == ND - 1))
                    yield
                nc.vector.tensor_scalar(
                    out=qkT[:, m, nh * 512:(nh + 1) * 512], in0=ps,
                    scalar1=bqkT[:, m:m + 1], scalar2=None, op0=ALU.add)
                yield

        def gen_v(sb0, sb1):
            """V rows for seq blocks [sb0, sb1): s-major with the softmax
            ones column per head."""
            for sb in range(sb0, sb1):
                for c0, cn in ((0, 512), (512, 256)):
                    ps = ps_mm.tile([P, 512], F32, tag="ps_mm",
                                    name=f"v{sb}_{c0}")
                    for kc in range(ND):
                        nc.tensor.matmul(ps[:, :cn],
                                         xT[:, kc, sb * P:(sb + 1) * P],
                                         w_bf[:, kc, 2 * D + c0:2 * D + c0 + cn],
                                         start=(kc == 0), stop=False)
                        yield
                    nc.tensor.matmul(ps[:, :cn], ones1, bv_bf[:, c0:c0 + cn],
                                     start=False, stop=True)
                    yield
                    h0 = c0 // HD
                    nh_h = cn // HD
                    vsrc = ps[:, :cn].rearrange("p (h c) -> p h c", c=HD)
                    vdst = vext[:, sb, :].rearrange("p (h c) -> p h c", c=65)
                    nc.vector.tensor_copy(vdst[:, h0:h0 + nh_h, 0:HD], vsrc)
                    yield

        ZROW = {(0, 0): 0, (0, 1): 32, (1, 0): 64, (1, 1): 96}

        def av_chain(g, half, qh, zb):
            """Full AV accumulation for (pair g, head half, q half) plus
            its epilogue: Z row staged into zb (partition row per combo),
            attn-out rows to aoT. Yields per PE instruction."""
            h = 2 * g + half
            qs = slice(qh * 512, (qh + 1) * 512)
            po = ps_av.tile([65, 512], F32, tag="ps_av", name=f"po{h}_{qh}")
            exps = stage_exps[g]
            for kb in range(NS):
                nc.tensor.matmul(po,
                                 vext[:, kb, h * 65:(h + 1) * 65],
                                 exps[half][kb][:, qs],
                                 start=(kb == 0), stop=(kb == NS - 1))
                yield
            rows = slice(half * HD, (half + 1) * HD)
            r = ZROW[(half, qh)]
            nc.vector.tensor_copy(zb[r:r + 1, :], po[64:65, :])
            nc.vector.tensor_copy(aoT[rows, g, qs], po[0:64, :])
            yield

        def av_and_norm(g, halves_qh):
            """AV accumulation for the given combos of pair g; once all
            four are in, one batched Ln+Exp(-x) on ScalarE produces 1/Z
            at partition rows {0,32,64,96}, broadcast with K=1 matmuls
            and applied with one DVE multiply per q half."""
            zb = sums_p.tile([97, 512], F32, tag="zb", bufs=2,
                             name=f"zb{g}")
            for half, qh in halves_qh:
                for _ in av_chain(g, half, qh, zb):
                    yield
            lnz = sums_p.tile([97, 512], F32, tag="lnz", bufs=2,
                              name=f"lnz{g}")
            rec = sums_p.tile([97, 512], BF, tag="rec", bufs=2,
                              name=f"rec{g}")
            nc.scalar.activation(lnz, zb, AF.Ln)
            nc.scalar.activation(rec, lnz, AF.Exp, scale=-1.0)
            yield
            for qh in range(2):
                qs = slice(qh * 512, (qh + 1) * 512)
                pb = ps_mm.tile([P, 512], F32, tag="ps_mm",
                                name=f"pb{g}_{qh}")
                for half in range(2):
                    rows = slice(half * HD, (half + 1) * HD)
                    r = ZROW[(half, qh)]
                    nc.tensor.matmul(pb[rows, :], ones4[r:r + 1, :],
                                     rec[r:r + 1, :], start=True, stop=True,
                                     tile_position=(r, half * HD))
                    yield
                nc.vector.tensor_mul(aoT[:, g, qs], aoT[:, g, qs], pb)
                yield

        def gen_proj_partial(sb_list):
            """proj partial sums over pairs 0-4 plus bias, drained bf16 to
            y01. Runs in stage 5's exp window (ps_mm pool is free of QKV
            work by then)."""
            for sb in sb_list:
                pj0 = ps_mm.tile([P, 512], F32, tag="ps_mm", name=f"pj0_{sb}")
                pj1 = ps_mm.tile([P, 512], F32, tag="ps_mm", name=f"pj1_{sb}")
                for g in range(ND - 1):
                    nc.tensor.matmul(pj0,
                                     aoT[:, g, sb * P:(sb + 1) * P],
                                     wp_bf[:, g, 0:512],
                                     start=(g == 0), stop=False)
                    yield
                    nc.tensor.matmul(pj1[:, 0:256],
                                     aoT[:, g, sb * P:(sb + 1) * P],
                                     wp_bf[:, g, 512:768],
                                     start=(g == 0), stop=False)
                    yield
                nc.tensor.matmul(pj0, ones1, bp_bf[:, 0:512],
                                 start=False, stop=True)
                nc.tensor.matmul(pj1[:, 0:256], ones1, bp_bf[:, 512:768],
                                 start=False, stop=True)
                yield
                nc.vector.tensor_copy(y01[:, sb, 0:512], pj0)
                nc.vector.tensor_copy(y01[:, sb, 512:768], pj1[:, 0:256])
                yield

        # ---------------- stages ----------------
        stage_exps = {}
        rzbs = {}

        def stage(g, fillers, n_fill=7, n_front=2):
            """Scores+exp for pair g; the first n_front kbs are emitted
            score-first so ScalarE never starves at stage boundaries;
            fillers are paced between the remaining kbs."""
            e0 = []
            e1 = []
            stage_exps[g] = (e0, e1)
            for kb in range(NS):
                if kb >= n_front:
                    for _ in range(n_fill):
                        if next(fillers, None) is None:
                            break
                ps0 = ps_sc.tile([P, S], F32, tag="ps_sc", name=f"sc0_{g}_{kb}")
                ps1 = ps_sc.tile([P, S], F32, tag="ps_sc", name=f"sc1_{g}_{kb}")
                for qh in range(2):
                    qs = slice(qh * 512, (qh + 1) * 512)
                    nc.tensor.matmul(ps0[:, qs],
                                     qkT[0:HD, ND + g, kb * P:(kb + 1) * P],
                                     qkT[0:HD, g, qs], start=True, stop=True)
                    nc.tensor.matmul(ps1[:, qs],
                                     qkT[HD:P, ND + g, kb * P:(kb + 1) * P],
                                     qkT[HD:P, g, qs], start=True, stop=True)
                t0 = expp.tile([P, S], BF, tag="expT", name=f"e0_{g}_{kb}")
                t1 = expp.tile([P, S], BF, tag="expT", name=f"e1_{g}_{kb}")
                nc.scalar.activation(t0, ps0, AF.Exp, scale=SCALE)
                nc.scalar.activation(t1, ps1, AF.Exp, scale=SCALE)
                e0.append(t0)
                e1.append(t1)
            return fillers

        ALL_COMBOS = ((0, 0), (0, 1), (1, 0), (1, 1))

        # emit pair-0 Q/K immediately (gated only on x + first W slices)
        for _ in gen_qkT(0):
            pass

        # stage 0: scores(0); fillers: V sb0-4 + qkT(1)
        f = stage(0, itertools.chain(gen_v(0, 5), gen_qkT(1)), n_fill=19)
        for _ in f:
            pass
        # stage 1: scores(1); fillers: V sb5-7 + qkT(2) + AV(0)
        # (V before the norm pb tiles: both use the 2-buf ps_mm ring, and
        # AV depends on V, so V's tiles must be allocated first)
        f = stage(1, itertools.chain(gen_v(5, NS), gen_qkT(2),
                                     av_and_norm(0, ALL_COMBOS)), n_fill=21)
        for _ in f:
            pass
        # stages 2-4: scores(g); fillers: qkT(g+1) + AV(g-1)
        for g in range(2, ND - 1):
            f = stage(g, itertools.chain(gen_qkT(g + 1),
                                         av_and_norm(g - 1, ALL_COMBOS)),
                      n_fill=13)
            for _ in f:
                pass
        # stage 5: scores(5); fillers: AV(4) + proj partials (pairs 0-4);
        # AV(5) runs in the post-loop drain (its exps only exist then)
        f = stage(ND - 1,
                  itertools.chain(av_and_norm(ND - 2, ALL_COMBOS),
                                  gen_proj_partial(range(NS)),
                                  av_and_norm(ND - 1, ALL_COMBOS)),
                  n_fill=20)
        for _ in f:
            pass

        # ---- tail: pair-5 proj contribution + final add + store ----
        for sb in range(NS):
            g = ND - 1
            pt0 = ps_mm.tile([P, 512], F32, tag="ps_mm", name=f"pt0_{sb}")
            pt1 = ps_mm.tile([P, 512], F32, tag="ps_mm", name=f"pt1_{sb}")
            nc.tensor.matmul(pt0, aoT[:, g, sb * P:(sb + 1) * P],
                             wp_bf[:, g, 0:512], start=True, stop=True)
            nc.tensor.matmul(pt1[:, 0:256], aoT[:, g, sb * P:(sb + 1) * P],
                             wp_bf[:, g, 512:768], start=True, stop=True)
            y_sb = yout.tile([P, D], F32, tag="y")
            nc.vector.tensor_tensor(y_sb[:, 0:512], pt0, y01[:, sb, 0:512],
                                    op=ALU.add)
            nc.vector.tensor_tensor(y_sb[:, 512:768], pt1[:, 0:256],
                                    y01[:, sb, 512:768], op=ALU.add)
            eng = nc.sync if sb % 2 == 0 else nc.scalar
            eng.dma_start(out_ext[sb * P:(sb + 1) * P, :], y_sb)


def kernel(**inputs):
    global _cached
    x = np.ascontiguousarray(np.asarray(inputs["x"], dtype=np.float32))
    w_qkv = np.ascontiguousarray(np.asarray(inputs["W_qkv"], dtype=np.float32))
    b_qkv = np.ascontiguousarray(np.asarray(inputs["b_qkv"], dtype=np.float32)).reshape(1, -1)
    w_proj = np.ascontiguousarray(np.asarray(inputs["W_proj"], dtype=np.float32))
    b_proj = np.ascontiguousarray(np.asarray(inputs["b_proj"], dtype=np.float32)).reshape(1, -1)

    if _cached is None:
        _cached = _build()
    nc = _cached

    in_maps = [{"x": x[b], "W_qkv": w_qkv, "b_qkv": b_qkv,
                "W_proj": w_proj, "b_proj": b_proj} for b in range(N_CORES)]
    last_err = None
    for _attempt in range(3):
        try:
            res = run_bass_kernel_spmd(nc, in_maps,
                                       core_ids=list(range(N_CORES)))
            return np.stack([res.results[i]["out"] for i in range(N_CORES)],
                            axis=0)
        except Exception as e:  # transient NRT device errors happen rarely
            last_err = e
            import time
            time.sleep(2.0)
    raise last_err
